# revision 3
# baseline (speedup 1.0000x reference)
"""Causal self-attention (B=4, T=2048, C=1024, H=16, D=64) on 8 TRN2 NeuronCores.

Sharding: 4 batches x 2 head-groups (8 heads each). Core c handles batch c//2,
heads 8*(c%2) .. 8*(c%2)+7. Host pre-transposes x and slices/transposes the
weights so the device kernel needs no on-chip transposes:

  phase 1:  qkT[feat, T] = Wqk_g @ x^T   (features on partitions)
            V[t, vfeat]  = x @ Wv_g^T    (keys on partitions, + ones column)
  phase 2:  S^T[k, q] = K_h Q_h^T tiles (fp32r matmuls, k on partitions),
            P^T = exp(S^T/8) * causal_mask,  out^T[d|sum, q] = [V_h|1]^T P^T,
            normalize with gpsimd partition-broadcast of 1/sum
  phase 3:  y = attn^T.T @ Wp_g^T slice, accumulated per 128-row block

Each core returns a [2048, 1024] partial; the host sums the two head-group
partials per batch.
"""

import numpy as np

T = 2048
N_CORES = 8

_CACHE = {}


def _build_module():
    from contextlib import ExitStack

    import concourse.tile as tile
    from concourse import bacc, mybir

    f32 = mybir.dt.float32
    f32r = mybir.dt.float32r
    Exp = mybir.ActivationFunctionType.Exp

    nc = bacc.Bacc("TRN2", target_bir_lowering=False, debug=False,
                   num_devices=N_CORES)

    xT_d = nc.dram_tensor("xT", (1024, 2048), f32r, kind="ExternalInput").ap()
    wqkT_d = nc.dram_tensor("wqkT", (1024, 1024), f32r, kind="ExternalInput").ap()
    wvT_d = nc.dram_tensor("wvT", (1024, 512), f32r, kind="ExternalInput").ap()
    wpT_d = nc.dram_tensor("wpT", (512, 1024), f32r, kind="ExternalInput").ap()
    mk_d = nc.dram_tensor("masks", (4, 128, 512), f32r, kind="ExternalInput").ap()
    y_d = nc.dram_tensor("y", (2048, 1024), f32, kind="ExternalOutput").ap()

    with tile.TileContext(nc) as tc, ExitStack() as ctx:
        pers = ctx.enter_context(tc.tile_pool(name="pers", bufs=1))
        sb_qT = pers.tile([128, 4, 2048], f32r, name="sb_qT")
        sb_kT = pers.tile([128, 4, 2048], f32r, name="sb_kT")
        sb_v = pers.tile([128, 16, 520], f32r, name="sb_v")
        v_view = sb_v[:].rearrange("p t (h e) -> p t h e", e=65)

        ps_big = ctx.enter_context(tc.tile_pool(name="ps_big", bufs=2, space="PSUM"))
        ps_s = ctx.enter_context(tc.tile_pool(name="ps_s", bufs=2, space="PSUM"))
        ps_o = ctx.enter_context(tc.tile_pool(name="ps_o", bufs=2, space="PSUM"))

        # ---------------- phase 1: qkv projections ----------------
        with ExitStack() as p1:
            ph1 = p1.enter_context(tc.tile_pool(name="ph1", bufs=1))
            wqk_pool = p1.enter_context(tc.tile_pool(name="wqk", bufs=3))

            sb_xT = ph1.tile([128, 8, 2048], f32r, name="sb_xT")
            xT_r = xT_d.rearrange("(co ci) t -> ci co t", ci=128)
            for co in range(8):
                nc.sync.dma_start(sb_xT[:, co, :], xT_r[:, co, :])

            sb_wvT = ph1.tile([128, 8, 512], f32r, name="sb_wvT")
            wvT_r = wvT_d.rearrange("(co ci) f -> ci co f", ci=128)
            for co in range(8):
                nc.sync.dma_start(sb_wvT[:, co, :], wvT_r[:, co, :])

            # ones column of sb_v via exp(0)=1 — also warms the ACT exp table
            zeros = ph1.tile([128, 128], f32, name="zeros")
            nc.vector.memset(zeros[:], 0.0)
            nc.scalar.activation(
                v_view[:, :, :, 64:65],
                zeros[:].rearrange("p (a b c) -> p a b c", a=16, b=8),
                Exp,
            )

            wqkT_r = wqkT_d.rearrange("(co ci) f -> ci co f", ci=128)
            for fb in [4, 5, 6, 7, 0, 1, 2, 3]:  # K features first, then Q
                wtile = wqk_pool.tile([128, 8, 128], f32r, tag="wqk")
                nc.sync.dma_start(wtile[:], wqkT_r[:, :, fb * 128:(fb + 1) * 128])
                dst, pblk = (sb_kT, fb - 4) if fb >= 4 else (sb_qT, fb)
                for tci in range(4):
                    ps = ps_big.tile([128, 512], f32, tag="psb")
                    for co in range(8):
                        nc.tensor.matmul(
                            ps[:],
                            lhsT=wtile[:, co, :],
                            rhs=sb_xT[:, co, tci * 512:(tci + 1) * 512],
                            start=(co == 0), stop=(co == 7),
                        )
                    nc.vector.tensor_copy(
                        dst[:, pblk, tci * 512:(tci + 1) * 512], ps[:])

            for tblk in range(16):
                ps = ps_big.tile([128, 512], f32, tag="psb")
                for co in range(8):
                    nc.tensor.matmul(
                        ps[:],
                        lhsT=sb_xT[:, co, tblk * 128:(tblk + 1) * 128],
                        rhs=sb_wvT[:, co, :],
                        start=(co == 0), stop=(co == 7),
                    )
                nc.vector.tensor_copy(
                    v_view[:, tblk, :, 0:64],
                    ps[:].rearrange("p (h d) -> p h d", d=64),
                )

        # ---------------- phase 2 + 3: attention + proj ----------------
        ph2 = ctx.enter_context(tc.tile_pool(name="ph2", bufs=1))
        exp_pool = ctx.enter_context(tc.tile_pool(name="expp", bufs=3))
        norm_pool = ctx.enter_context(tc.tile_pool(name="normp", bufs=2))
        y_pool = ctx.enter_context(tc.tile_pool(name="yp", bufs=2))

        sb_attnT = ph2.tile([128, 4, 2048], f32r, name="sb_attnT")
        sb_wpT = ph2.tile([128, 4, 1024], f32r, name="sb_wpT")
        wpT_r = wpT_d.rearrange("(ko ki) n -> ki ko n", ki=128)
        for ko in range(4):
            nc.sync.dma_start(sb_wpT[:, ko, :], wpT_r[:, ko, :])
        sb_masks = ph2.tile([128, 4, 512], f32r, name="sb_masks")
        for i in range(4):
            nc.sync.dma_start(sb_masks[:, i, :], mk_d[i])

        for qc in range(4):
            for h in range(8):
                p_, rr = h // 2, (h % 2) * 64
                nblk = 4 * qc + 4
                po = ps_o.tile([65, 512], f32, tag="pso")
                for jg in range(nblk // 2):
                    pss = ps_s.tile([128, 2, 512], f32, tag="pss")
                    for jj in range(2):
                        j = jg * 2 + jj
                        nc.tensor.matmul(
                            pss[:, jj, :],
                            lhsT=sb_kT[rr:rr + 64, p_, j * 128:(j + 1) * 128],
                            rhs=sb_qT[rr:rr + 64, p_, qc * 512:(qc + 1) * 512],
                            start=True, stop=True,
                        )
                    et = exp_pool.tile([128, 2, 512], f32r, tag="expT")
                    nc.scalar.activation(et[:], pss[:], Exp, scale=0.125)
                    for jj in range(2):
                        j = jg * 2 + jj
                        if j >= 4 * qc:
                            nc.vector.tensor_mul(
                                et[:, jj, :], et[:, jj, :],
                                sb_masks[:, j - 4 * qc, :])
                        nc.tensor.matmul(
                            po[:],
                            lhsT=v_view[:, j, h, :],
                            rhs=et[:, jj, :],
                            start=(j == 0), stop=(j == nblk - 1),
                        )
                sums = norm_pool.tile([1, 512], f32, tag="sums")
                nc.vector.tensor_copy(sums[:], po[64:65, :])
                recip = norm_pool.tile([1, 512], f32, tag="recip")
                nc.vector.reciprocal(recip[:], sums[:])
                bcast = norm_pool.tile([64, 512], f32, tag="bcast")
                nc.gpsimd.partition_broadcast(bcast[:], recip[:])
                nc.vector.tensor_mul(
                    sb_attnT[rr:rr + 64, p_, qc * 512:(qc + 1) * 512],
                    po[0:64, :], bcast[:])

            for tblk in range(qc * 4, qc * 4 + 4):
                ysb = y_pool.tile([128, 1024], f32, tag="ysb")
                for n in range(2):
                    pj = ps_big.tile([128, 512], f32, tag="psb")
                    for ko in range(4):
                        nc.tensor.matmul(
                            pj[:],
                            lhsT=sb_attnT[:, ko, tblk * 128:(tblk + 1) * 128],
                            rhs=sb_wpT[:, ko, n * 512:(n + 1) * 512],
                            start=(ko == 0), stop=(ko == 3),
                        )
                    nc.vector.tensor_copy(ysb[:, n * 512:(n + 1) * 512], pj[:])
                nc.sync.dma_start(y_d[tblk * 128:(tblk + 1) * 128, :], ysb[:])

    nc.compile()
    return nc


def _get_module():
    if "nc" not in _CACHE:
        _CACHE["nc"] = _build_module()
    return _CACHE["nc"]


def _make_masks():
    # masks[i][kk, q] = 1 iff q >= i*128 + kk  (q, kk local to a 512/128 chunk)
    q = np.arange(512)[None, :]
    kk = np.arange(128)[:, None]
    return np.stack([(q >= i * 128 + kk) for i in range(4)]).astype(np.float32)


def make_in_maps(x, W_qkv, W_proj):
    x = np.asarray(x, dtype=np.float32)
    W_qkv = np.asarray(W_qkv, dtype=np.float32)
    W_proj = np.asarray(W_proj, dtype=np.float32)
    masks = _make_masks()
    in_maps = []
    for c in range(N_CORES):
        b, g = c // 2, c % 2
        s = 512 * g
        wqk = np.concatenate([W_qkv[s:s + 512], W_qkv[1024 + s:1024 + s + 512]], 0)
        in_maps.append({
            "xT": np.ascontiguousarray(x[b].T),
            "wqkT": np.ascontiguousarray(wqk.T),
            "wvT": np.ascontiguousarray(W_qkv[2048 + s:2048 + s + 512].T),
            "wpT": np.ascontiguousarray(W_proj[:, s:s + 512].T),
            "masks": masks,
        })
    return in_maps


def run(x, W_qkv, W_proj, trace=False):
    """Returns (y_full [4,2048,1024], BassKernelResults)."""
    from concourse import bass_utils

    nc = _get_module()
    in_maps = make_in_maps(x, W_qkv, W_proj)
    res = bass_utils.run_bass_kernel_spmd(
        nc, in_maps, core_ids=list(range(N_CORES)), trace=trace)
    y = np.zeros((4, T, 1024), np.float32)
    for b in range(4):
        y[b] = res.results[2 * b]["y"] + res.results[2 * b + 1]["y"]
    return y, res


def kernel(x, W_qkv, W_proj):
    y, _ = run(x, W_qkv, W_proj, trace=False)
    return y


# revision 7
# speedup vs baseline: 1.2359x; 1.2359x over previous
"""Causal self-attention (B=4, T=2048, C=1024, H=16, D=64) on 8 TRN2 NeuronCores.

Sharding: 4 batches x 2 head-groups (8 heads each). Core c handles batch c//2,
heads 8*(c%2) .. 8*(c%2)+7. Host pre-transposes x and slices/transposes the
weights so the device kernel needs no on-chip transposes:

  phase 1:  qkT[feat, T] = Wqk_g @ x^T   (features on partitions)
            V[t, vfeat]  = x @ Wv_g^T    (keys on partitions, + ones column)
  phase 2:  S^T[k, q] = K_h Q_h^T tiles (fp32r matmuls, k on partitions),
            P^T = exp(S^T/8) with causal zero/tri masking,
            out^T[d|sum, q] = [V_h|1]^T P^T, normalized by 1/sum via
            approx-reciprocal + DMA partition-broadcast
  phase 3:  y = attn^T.T @ Wp_g^T slice, interleaved with attention as PE filler

Each core returns a [2048, 1024] partial; the host sums the two head-group
partials per batch.
"""

import numpy as np

T = 2048
N_CORES = 8

_CACHE = {}


def _build_module():
    from contextlib import ExitStack

    import concourse.tile as tile
    from concourse import bacc, mybir

    f32 = mybir.dt.float32
    f32r = mybir.dt.float32r
    Exp = mybir.ActivationFunctionType.Exp

    nc = bacc.Bacc("TRN2", target_bir_lowering=False, debug=False,
                   num_devices=N_CORES)

    xT_d = nc.dram_tensor("xT", (1024, 2048), f32r, kind="ExternalInput").ap()
    wqkT_d = nc.dram_tensor("wqkT", (1024, 1024), f32r, kind="ExternalInput").ap()
    wvT_d = nc.dram_tensor("wvT", (1024, 512), f32r, kind="ExternalInput").ap()
    wpT_d = nc.dram_tensor("wpT", (512, 1024), f32r, kind="ExternalInput").ap()
    mk_d = nc.dram_tensor("masks", (4, 128, 512), f32r, kind="ExternalInput").ap()
    y_d = nc.dram_tensor("y", (2048, 1024), f32, kind="ExternalOutput").ap()

    with tile.TileContext(nc) as tc, ExitStack() as ctx:
        pers = ctx.enter_context(tc.tile_pool(name="pers", bufs=1))
        sb_qT = pers.tile([128, 4, 2048], f32r, name="sb_qT")
        sb_kT = pers.tile([128, 4, 2048], f32r, name="sb_kT")
        sb_v = pers.tile([128, 16, 520], f32r, name="sb_v")
        v_view = sb_v[:].rearrange("p t (h e) -> p t h e", e=65)

        ps_big = ctx.enter_context(tc.tile_pool(name="ps_big", bufs=2, space="PSUM"))
        ps_s = ctx.enter_context(tc.tile_pool(name="ps_s", bufs=2, space="PSUM"))
        ps_o = ctx.enter_context(tc.tile_pool(name="ps_o", bufs=2, space="PSUM"))

        # ---------------- phase 1: qkv projections ----------------
        with ExitStack() as p1:
            ph1 = p1.enter_context(tc.tile_pool(name="ph1", bufs=1))
            wqk_pool = p1.enter_context(tc.tile_pool(name="wqk", bufs=3))

            sb_xT = ph1.tile([128, 8, 2048], f32r, name="sb_xT")
            xT_r = xT_d.rearrange("(co ci) t -> ci co t", ci=128)
            # chunked by t so the first matmuls only wait for the first 2MB
            for tci in range(4):
                for co in range(8):
                    nc.sync.dma_start(
                        sb_xT[:, co, tci * 512:(tci + 1) * 512],
                        xT_r[:, co, tci * 512:(tci + 1) * 512])

            sb_wvT = ph1.tile([128, 8, 512], f32r, name="sb_wvT")
            wvT_r = wvT_d.rearrange("(co ci) f -> ci co f", ci=128)
            for co in range(8):
                nc.sync.dma_start(sb_wvT[:, co, :], wvT_r[:, co, :])

            # ones column of sb_v via exp(0)=1 — also warms the ACT exp table
            zeros = ph1.tile([128, 128], f32, name="zeros")
            nc.vector.memset(zeros[:], 0.0)
            nc.scalar.activation(
                v_view[:, :, :, 64:65],
                zeros[:].rearrange("p (a b c) -> p a b c", a=16, b=8),
                Exp,
            )

            wqkT_r = wqkT_d.rearrange("(co ci) f -> ci co f", ci=128)
            for fb in [4, 5, 6, 7, 0, 1, 2, 3]:  # K features first, then Q
                wtile = wqk_pool.tile([128, 8, 128], f32r, tag="wqk")
                nc.sync.dma_start(wtile[:], wqkT_r[:, :, fb * 128:(fb + 1) * 128])
                dst, pblk = (sb_kT, fb - 4) if fb >= 4 else (sb_qT, fb)
                for tci in range(4):
                    ps = ps_big.tile([128, 512], f32, tag="psb")
                    for co in range(8):
                        nc.tensor.matmul(
                            ps[:],
                            lhsT=wtile[:, co, :],
                            rhs=sb_xT[:, co, tci * 512:(tci + 1) * 512],
                            start=(co == 0), stop=(co == 7),
                        )
                    nc.vector.tensor_copy(
                        dst[:, pblk, tci * 512:(tci + 1) * 512], ps[:])

            for tblk in range(16):
                ps = ps_big.tile([128, 512], f32, tag="psb")
                for co in range(8):
                    nc.tensor.matmul(
                        ps[:],
                        lhsT=sb_xT[:, co, tblk * 128:(tblk + 1) * 128],
                        rhs=sb_wvT[:, co, :],
                        start=(co == 0), stop=(co == 7),
                    )
                nc.vector.tensor_copy(
                    v_view[:, tblk, :, 0:64],
                    ps[:].rearrange("p (h d) -> p h d", d=64),
                )

        # ---------------- phase 2 + 3: attention + proj ----------------
        ph2 = ctx.enter_context(tc.tile_pool(name="ph2", bufs=1))
        exp_pool = ctx.enter_context(tc.tile_pool(name="expp", bufs=3))
        norm_pool = ctx.enter_context(tc.tile_pool(name="normp", bufs=3))
        y_pool = ctx.enter_context(tc.tile_pool(name="yp", bufs=2))

        sb_attnT = ph2.tile([128, 4, 2048], f32r, name="sb_attnT")
        sb_wpT = ph2.tile([128, 4, 1024], f32r, name="sb_wpT")
        wpT_r = wpT_d.rearrange("(ko ki) n -> ki ko n", ki=128)
        for ko in range(4):
            nc.sync.dma_start(sb_wpT[:, ko, :], wpT_r[:, ko, :])
        sb_masks = ph2.tile([128, 4, 512], f32r, name="sb_masks")
        for i in range(4):
            nc.sync.dma_start(sb_masks[:, i, :], mk_d[i])

        def emit_proj(tblk):
            ysb = y_pool.tile([128, 1024], f32, tag="ysb")
            for n in range(2):
                pj = ps_big.tile([128, 512], f32, tag="psb")
                for ko in range(4):
                    nc.tensor.matmul(
                        pj[:],
                        lhsT=sb_attnT[:, ko, tblk * 128:(tblk + 1) * 128],
                        rhs=sb_wpT[:, ko, n * 512:(n + 1) * 512],
                        start=(ko == 0), stop=(ko == 3),
                    )
                nc.vector.tensor_copy(ysb[:, n * 512:(n + 1) * 512], pj[:])
            nc.sync.dma_start(y_d[tblk * 128:(tblk + 1) * 128, :], ysb[:])

        for qc in range(4):
            for h in range(8):
                p_, rr = h // 2, (h % 2) * 64
                nblk = 4 * qc + 4
                po = ps_o.tile([65, 512], f32, tag="pso")
                for jg in range(nblk // 2):
                    pss = ps_s.tile([128, 2, 512], f32, tag="pss")
                    for jj in range(2):
                        j = jg * 2 + jj
                        nc.tensor.matmul(
                            pss[:, jj, :],
                            lhsT=sb_kT[rr:rr + 64, p_, j * 128:(j + 1) * 128],
                            rhs=sb_qT[rr:rr + 64, p_, qc * 512:(qc + 1) * 512],
                            start=True, stop=True,
                        )
                    et = exp_pool.tile([128, 2, 512], f32r, tag="expT")
                    nc.scalar.activation(et[:], pss[:], Exp, scale=0.125)
                    for jj in range(2):
                        j = jg * 2 + jj
                        if j >= 4 * qc:
                            i = j - 4 * qc
                            w = (i + 1) * 128
                            nc.vector.tensor_mul(
                                et[:, jj, 0:w], et[:, jj, 0:w],
                                sb_masks[:, i, 0:w])
                        nc.tensor.matmul(
                            po[:],
                            lhsT=v_view[:, j, h, :],
                            rhs=et[:, jj, :],
                            start=(j == 0), stop=(j == nblk - 1),
                        )
                # free po quickly: pull numerators + sums out, then normalize
                att_slice = sb_attnT[rr:rr + 64, p_, qc * 512:(qc + 1) * 512]
                nc.vector.tensor_copy(att_slice, po[0:64, :])
                sums = norm_pool.tile([1, 512], f32, tag="sums")
                nc.vector.tensor_copy(sums[:], po[64:65, :])
                recip = norm_pool.tile([1, 512], f32, tag="recip")
                nc.vector.reciprocal_approx_fast(out=recip[:], in_=sums[:])
                bcast = norm_pool.tile([128, 512], f32, tag="bcast")
                nc.gpsimd.partition_broadcast(bcast[:], recip[:])
                nc.vector.tensor_mul(att_slice, att_slice, bcast[rr:rr + 64, :])
                if qc > 0 and h % 2 == 1:
                    emit_proj((qc - 1) * 4 + h // 2)
            if qc == 3:
                for tblk in range(12, 16):
                    emit_proj(tblk)

    nc.compile()
    return nc


def _get_module():
    if "nc" not in _CACHE:
        _CACHE["nc"] = _build_module()
    return _CACHE["nc"]


def _make_masks():
    # masks[i][kk, q] = 1 iff q >= i*128 + kk  (q local to the 512 chunk)
    q = np.arange(512)[None, :]
    kk = np.arange(128)[:, None]
    return np.stack([(q >= i * 128 + kk) for i in range(4)]).astype(np.float32)


def make_in_maps(x, W_qkv, W_proj):
    x = np.asarray(x, dtype=np.float32)
    W_qkv = np.asarray(W_qkv, dtype=np.float32)
    W_proj = np.asarray(W_proj, dtype=np.float32)
    masks = _make_masks()
    in_maps = []
    for c in range(N_CORES):
        b, g = c // 2, c % 2
        s = 512 * g
        wqk = np.concatenate([W_qkv[s:s + 512], W_qkv[1024 + s:1024 + s + 512]], 0)
        in_maps.append({
            "xT": np.ascontiguousarray(x[b].T),
            "wqkT": np.ascontiguousarray(wqk.T),
            "wvT": np.ascontiguousarray(W_qkv[2048 + s:2048 + s + 512].T),
            "wpT": np.ascontiguousarray(W_proj[:, s:s + 512].T),
            "masks": masks,
        })
    return in_maps


def run(x, W_qkv, W_proj, trace=False):
    """Returns (y_full [4,2048,1024], BassKernelResults)."""
    from concourse import bass_utils

    nc = _get_module()
    in_maps = make_in_maps(x, W_qkv, W_proj)
    res = bass_utils.run_bass_kernel_spmd(
        nc, in_maps, core_ids=list(range(N_CORES)), trace=trace)
    y = np.zeros((4, T, 1024), np.float32)
    for b in range(4):
        y[b] = res.results[2 * b]["y"] + res.results[2 * b + 1]["y"]
    return y, res


def kernel(x, W_qkv, W_proj):
    y, _ = run(x, W_qkv, W_proj, trace=False)
    return y
